# revision 13
# baseline (speedup 1.0000x reference)
"""Multi-head attention (SEQ=4096, d_model=1024, 16 heads of d=64) on 8 TRN2
NeuronCores, tensor-parallel over heads (2 heads/core).

v5 design (vs the 475us v1 baseline):
  1. Heads PAIRED per q-chunk. Scores stay K=128 zero-padded per head
     (khT2z[h]): K=64 or narrow-M matmuls never unthrottle the PE HAM
     activity clock gate (measured 59% of a run at 1.2 GHz with K=64 row
     tiles), so the 2x padded contraction at full clock beats the
     "efficient" half-size matmul. qhT2 [128, 4096] holds head0 in
     partitions 0-63, head1 in 64-127; khT2z[h] has the other head's
     partition range memset to 1e-30 (true zeros slow the PE).
     AV stays bf16: fp8 probs measured 3.4-6.9% attention error (softmax
     -weighted reductions do NOT average per-element quant noise), over
     the harness gate.
  2. q/k/v DMA'd in interleaved 512-column groups, each projected into
     one transient PSUM tile and drained immediately -- the PE trickles
     behind the DMA stream with no long idle (v1/v3 idled 16us waiting
     for serially-loaded tensors and went HAM-cold).
  3. Producer-side softmax normalization: sums row (from the ones column
     of vh2) -> ScalarE copy to partition 0 -> DVE reciprocal_approx_fast
     -> K=1 PE broadcast matmul -> drain multiply. No sums AllToAll, no
     post-collective scale machinery. The reciprocal chain of each
     q-chunk is deferred past the next q-chunk's first score pair so the
     PE never waits on it.
  4. Output re-shard via TWO half AllToAlls (collectives here cost ~15us
     fixed regardless of size, so fewer+bigger wins): half a after qc3
     (hidden under qc4-7), half b after qc7 covered by the deferred
     first-half FC matmuls. Received block j = heads {2j,2j+1}, so Wfc
     needs no permutation. Core c owns q rows {2048a + 256c .. +256}.
  exp split: head0 tiles on ScalarE LUT, head1 on VectorE Schraudolph
  bf16 bit-trick (sawtooth cancels under softmax renormalization).
"""

import os
import sys

sys.path.insert(0, "/opt/trn_rl_repo")

import numpy as np
import ml_dtypes

import concourse.bass as bass
import concourse.mybir as mybir
import concourse.tile as tile
from concourse import bacc
from concourse.bass_utils import run_bass_kernel_spmd

SEQ = 4096
DM = 1024
NH = 16
DK = 64
DV = 64
CORES = 8
P = 128
HL = 2 * DK  # 128: two heads' head-dim per core
SROWS = SEQ // CORES  # 512 output rows per core
MO = DM // P  # 8 m-chunks of d_model
KB = SEQ // P  # 32 key blocks
QCS = 8  # q chunks
QCW = SEQ // QCS  # 512
NHALF = 2  # A2A halves (4 qchunks each)
HW = SEQ // NHALF  # 2048 columns per half
MSG = HW // CORES  # 256 columns per message
F32 = mybir.dt.float32
BF16 = mybir.dt.bfloat16

SCALE = 1.0 / float(np.sqrt(DK))
EXP_A = 128.0 / float(np.log(2.0))  # bf16-bits Schraudolph slope
EXP_B = 16256.0 - 5.5  # 127*128 - C


def build():
    nc = bacc.Bacc(
        "TRN2",
        target_bir_lowering=False,
        debug=False,
        enable_asserts=True,
        num_devices=CORES,
    )

    qT = nc.dram_tensor("qT", [DM, SEQ], BF16, kind="ExternalInput").ap()
    kT = nc.dram_tensor("kT", [DM, SEQ], BF16, kind="ExternalInput").ap()
    vT = nc.dram_tensor("vT", [DM, SEQ], BF16, kind="ExternalInput").ap()
    wqT = nc.dram_tensor("wqT", [DM, HL], BF16, kind="ExternalInput").ap()
    wkT = nc.dram_tensor("wkT", [DM, HL], BF16, kind="ExternalInput").ap()
    wvT = nc.dram_tensor("wvT", [DM, HL], BF16, kind="ExternalInput").ap()
    wfcT = nc.dram_tensor("wfcT", [DM, DM], BF16, kind="ExternalInput").ap()
    qres = nc.dram_tensor("qres", [SROWS, DM], F32, kind="ExternalInput").ap()
    out = nc.dram_tensor("out", [SROWS, DM], F32, kind="ExternalOutput").ap()

    qT_r = qT.rearrange("(o p) s -> p o s", p=P)
    kT_r = kT.rearrange("(o p) s -> p o s", p=P)
    vT_r = vT.rearrange("(o p) s -> p o s", p=P)
    wqT_r = wqT.rearrange("(o p) h -> p o h", p=P)
    wkT_r = wkT.rearrange("(o p) h -> p o h", p=P)
    wvT_r = wvT.rearrange("(o p) h -> p o h", p=P)
    wfcT_r = wfcT.rearrange("(o p) d -> p o d", p=P)
    qres_r = qres.rearrange("(b p) d -> p b d", p=P)
    out_r = out.rearrange("(b p) d -> p b d", p=P)

    with tile.TileContext(nc) as tc:
        with (
            tc.tile_pool(name="const", bufs=1) as cpool,
            tc.tile_pool(name="xin", bufs=7) as xpool,
            tc.tile_pool(name="pt", bufs=5) as ptpool,
            tc.tile_pool(name="small", bufs=2) as spool,
            tc.tile_pool(name="rr", bufs=2) as rpool,
            tc.tile_pool(name="qrp", bufs=1) as qrpool,
            tc.tile_pool(name="ps", bufs=8, space="PSUM") as ps,
            tc.tile_pool(name="dram", bufs=1, space="DRAM") as dr,
        ):
            # ---- persistent tiles ----
            wq_sb = cpool.tile([P, MO, HL], BF16, tag="wq")
            wk_sb = cpool.tile([P, MO, HL], BF16, tag="wk")
            wv_sb = cpool.tile([P, MO, HL], BF16, tag="wv")
            nc.sync.dma_start(wq_sb[:], wqT_r[:])
            nc.gpsimd.dma_start(wk_sb[:], wkT_r[:])
            nc.sync.dma_start(wv_sb[:], wvT_r[:])

            qhT2 = cpool.tile([P, SEQ], BF16, tag="qhT2")
            khT2z = [
                cpool.tile([P, SEQ], BF16, tag=f"khT2z{h}", name=f"khT2z{h}")
                for h in range(2)
            ]
            # ~1e-30 instead of 0.0: zero-valued weights measurably slow the
            # PE (value-dependent activity gating); contributes ~1e-30 * O(1)
            # to the fp32 scores -- negligible.
            nc.vector.memset(khT2z[0][DK:P, :], 1e-30)
            nc.vector.memset(khT2z[1][0:DK, :], 1e-30)
            outT = cpool.tile([P, SEQ], BF16, tag="outT")
            # vh2[:, b, h, 0:64] = head-h V rows for key block b; col 64 = ones
            vh2 = cpool.tile([P, KB, 2, DV + 1], BF16, tag="vh2")
            nc.vector.memset(vh2[:, :, :, DV : DV + 1], 1.0)
            ones1 = cpool.tile([1, DV], F32, tag="ones1")
            nc.vector.memset(ones1[:], 1.0)

            # ---- phase 1: q/k/v projections, interleaved column groups ----
            # round g: qg + kg + vg [128, 8, 512] each; 8 accumulating MMs
            # per projection into a transient PSUM tile, drained immediately.
            xg = {}
            for g in range(MO):
                for nm, src in (("q", qT_r), ("k", kT_r), ("v", vT_r)):
                    xt = xpool.tile([P, MO, QCW], BF16, tag="xin",
                                    name=f"{nm}g{g}")
                    eng = {
                        "q": nc.sync,
                        "k": nc.gpsimd,
                        "v": nc.sync if g % 2 == 0 else nc.gpsimd,
                    }[nm]
                    eng.dma_start(xt[:], src[:, :, g * QCW : (g + 1) * QCW])
                    xg[(nm, g)] = xt
            for g in range(MO):
                gs = slice(g * QCW, (g + 1) * QCW)
                pq = ps.tile([P, QCW], F32, tag="ps", name=f"pq{g}")
                for o in range(MO):
                    nc.tensor.matmul(
                        pq[:], wq_sb[:, o, :], xg[("q", g)][:, o, :],
                        start=(o == 0), stop=(o == MO - 1),
                    )
                if g % 2 == 0:
                    nc.scalar.copy(out=qhT2[:, gs], in_=pq[:])
                else:
                    nc.vector.tensor_copy(out=qhT2[:, gs], in_=pq[:])
                pk = ps.tile([P, QCW], F32, tag="ps", name=f"pk{g}")
                for o in range(MO):
                    nc.tensor.matmul(
                        pk[:], wk_sb[:, o, :], xg[("k", g)][:, o, :],
                        start=(o == 0), stop=(o == MO - 1),
                    )
                nc.scalar.copy(out=khT2z[0][0:DK, gs], in_=pk[0:DK])
                nc.vector.tensor_copy(out=khT2z[1][DK:P, gs], in_=pk[DK:P])
                for b4 in range(QCW // P):
                    b = g * (QCW // P) + b4
                    pv = ps.tile([P, HL], F32, tag="ps")
                    for o in range(MO):
                        nc.tensor.matmul(
                            pv[:],
                            xg[("v", g)][:, o, b4 * P : (b4 + 1) * P],
                            wv_sb[:, o, :],
                            start=(o == 0),
                            stop=(o == MO - 1),
                        )
                    nc.scalar.copy(out=vh2[:, b, 0, 0:DV], in_=pv[:, 0:DK])
                    nc.vector.tensor_copy(
                        out=vh2[:, b, 1, 0:DV], in_=pv[:, DK:HL]
                    )

            # late constants: after projection loads so they don't delay them
            wfc_sb = cpool.tile([P, MO, DM], BF16, tag="wfc")
            nc.gpsimd.dma_start(wfc_sb[:], wfcT_r[:])
            qres_sb = qrpool.tile([P, SROWS // P, DM], F32, tag="qre")
            nc.gpsimd.dma_start(qres_sb[:], qres_r[:])

            # ---- phase 2: attention, heads paired; phase 3 interleaved ----
            dvos = []

            def emit_fc_half(a):
                """FC for sb-blocks {2a, 2a+1} (q rows of A2A half a)."""
                ofk = cpool.tile([P, MO, MSG], BF16, tag=f"of{a}", name=f"of{a}")
                nc.sync.dma_start(
                    ofk[:], dvos[a].rearrange("(o p) s -> p o s", p=P)
                )
                for sb in (2 * a, 2 * a + 1):
                    for nm in range(DM // QCW):
                        pf = ps.tile([P, QCW], F32, tag="ps",
                                     name=f"pf{sb}_{nm}")
                        for o in range(MO):
                            nc.tensor.matmul(
                                pf[:],
                                ofk[:, o, (sb % 2) * P : (sb % 2 + 1) * P],
                                wfc_sb[:, o, nm * QCW : (nm + 1) * QCW],
                                start=(o == 0),
                                stop=(o == MO - 1),
                            )
                        eo = spool.tile([P, QCW], F32, tag="eo")
                        nc.scalar.activation(
                            out=eo[:], in_=pf[:],
                            func=mybir.ActivationFunctionType.Relu,
                        )
                        nc.gpsimd.tensor_add(
                            out=eo[:],
                            in0=eo[:],
                            in1=qres_sb[:, sb, nm * QCW : (nm + 1) * QCW],
                        )
                        nc.sync.dma_start(
                            out_r[:, sb, nm * QCW : (nm + 1) * QCW], eo[:]
                        )

            def emit_block(qc, b, avT):
                """Scores + exp + AV for both heads, key block b."""
                q0 = qc * QCW
                sco = [
                    ps.tile([P, QCW], F32, tag="ps", name=f"sco{h}")
                    for h in range(2)
                ]
                pt = [
                    ptpool.tile([P, QCW], BF16, tag="pt", name=f"pt{h}")
                    for h in range(2)
                ]
                for h in range(2):
                    nc.tensor.matmul(
                        sco[h][:],
                        khT2z[h][:, b * P : (b + 1) * P],
                        qhT2[:, q0 : q0 + QCW],
                        start=True,
                        stop=True,
                    )
                # exp: head0 on ScalarE LUT, head1 on VectorE bit-trick
                nc.scalar.activation(
                    out=pt[0][:], in_=sco[0][:],
                    func=mybir.ActivationFunctionType.Exp,
                    scale=float(SCALE),
                )
                nc.vector.tensor_scalar(
                    out=pt[1][:].bitcast(mybir.dt.int16),
                    in0=sco[1][:],
                    scalar1=float(SCALE * EXP_A),
                    scalar2=float(EXP_B),
                    op0=mybir.AluOpType.mult,
                    op1=mybir.AluOpType.add,
                )
                for h in range(2):
                    nc.tensor.matmul(
                        avT[h][:],
                        vh2[:, b, h, :],
                        pt[h][:],
                        start=(b == 0),
                        stop=(b == KB - 1),
                    )

            def drain_stage1(avT):
                """Per-head: sums row -> partition-0 stage -> reciprocal.
                ScalarE/VectorE only; runs under the next q-chunk's scores."""
                rs = []
                for h in range(2):
                    s0 = rpool.tile([1, QCW], F32, tag="s0", name=f"s0{h}")
                    nc.scalar.copy(out=s0[:], in_=avT[h][DV : DV + 1, :])
                    r_sb = rpool.tile([1, QCW], F32, tag="r", name=f"r{h}")
                    nc.vector.reciprocal_approx_fast(out=r_sb[:], in_=s0[:])
                    rs.append(r_sb)
                return rs

            def drain_stage2(qc, avT, rs):
                """K=1 broadcast matmuls + normalize-multiply into outT."""
                q0 = qc * QCW
                for h in range(2):
                    hs = h * DK
                    bct = ps.tile([DV, QCW], F32, tag="ps", name=f"bct{h}")
                    nc.tensor.matmul(
                        bct[:], ones1[:], rs[h][:], start=True, stop=True
                    )
                    bc_sb = rpool.tile([DV, QCW], BF16, tag="bc", name=f"bc{h}")
                    nc.scalar.copy(out=bc_sb[:], in_=bct[:])
                    nc.vector.tensor_mul(
                        out=outT[hs : hs + DK, q0 : q0 + QCW],
                        in0=avT[h][0:DV, :],
                        in1=bc_sb[:],
                    )

            def emit_a2a(a):
                dvi = dr.tile([CORES * P, MSG], BF16, name=f"a2ai{a}")
                dvo = dr.tile([CORES * P, MSG], BF16, name=f"a2ao{a}")
                dvos.append(dvo)
                for j in range(CORES):
                    nc.sync.dma_start(
                        dvi[j * P : (j + 1) * P, :],
                        outT[:, HW * a + j * MSG :][:, :MSG],
                    )
                nc.gpsimd.collective_compute(
                    "AllToAll",
                    mybir.AluOpType.bypass,
                    replica_groups=[list(range(CORES))],
                    ins=[dvi.opt()],
                    outs=[dvo.opt()],
                )

            pending = None  # (qc, avT, rs) whose stage2 is deferred
            for qc in range(QCS):
                avT = [
                    ps.tile([DV + 1, QCW], F32, tag="ps", name=f"avT{h}")
                    for h in range(2)
                ]
                for b in range(KB):
                    emit_block(qc, b, avT)
                    if b == 1 and pending is not None:
                        drain_stage2(*pending)
                        pqc = pending[0]
                        pending = None
                        if pqc % (QCS // NHALF) == QCS // NHALF - 1:
                            a = pqc // (QCS // NHALF)
                            emit_a2a(a)
                            if a >= 1:
                                emit_fc_half(a - 1)
                rs = drain_stage1(avT)
                pending = (qc, avT, rs)
            drain_stage2(*pending)
            emit_a2a(NHALF - 1)
            emit_fc_half(NHALF - 2)
            emit_fc_half(NHALF - 1)

    nc.compile()
    return nc


def make_in_maps(q, k, v, Wq, Wk, Wv, Wfc):
    bf = ml_dtypes.bfloat16
    qT = np.ascontiguousarray(q.T).astype(bf)
    kT = np.ascontiguousarray(k.T).astype(bf)
    vT = np.ascontiguousarray(v.T).astype(bf)
    wfcT = np.ascontiguousarray(Wfc.T).astype(bf)
    in_maps = []
    for c in range(CORES):
        sl = slice(c * HL, (c + 1) * HL)
        # core c owns q rows {2048a + 256c .. +256} for a=0,1
        rows = np.concatenate(
            [np.arange(HW * a + MSG * c, HW * a + MSG * (c + 1))
             for a in range(NHALF)]
        )
        in_maps.append(
            {
                "qT": qT,
                "kT": kT,
                "vT": vT,
                "wqT": np.ascontiguousarray(Wq[sl].T).astype(bf),
                "wkT": np.ascontiguousarray(Wk[sl].T).astype(bf),
                "wvT": np.ascontiguousarray(Wv[sl].T).astype(bf),
                "wfcT": wfcT,
                "qres": np.ascontiguousarray(q[rows]).astype(np.float32),
            }
        )
    return in_maps


_NC_CACHE = {}


def kernel(q, k, v, Wq, Wk, Wv, Wfc):
    key = "full"
    if key not in _NC_CACHE:
        _NC_CACHE[key] = build()
    nc = _NC_CACHE[key]
    in_maps = make_in_maps(q, k, v, Wq, Wk, Wv, Wfc)
    trace = bool(int(os.environ.get("KERNEL_TRACE", "0")))
    tc_env = os.environ.get("KERNEL_TRACE_CORES", "")
    kw = {}
    if tc_env:
        kw["trace_cores"] = [int(x) for x in tc_env.split(",")]
    res = run_bass_kernel_spmd(nc, in_maps, list(range(CORES)), trace=trace, **kw)
    if trace:
        kernel.last_exec_time_ns = res.exec_time_ns
        kernel.last_profile = res
    out = np.empty((SEQ, DM), dtype=np.float32)
    for c in range(CORES):
        rc = res.results[c]["out"]
        for a in range(NHALF):
            out[HW * a + MSG * c : HW * a + MSG * (c + 1)] = rc[
                a * MSG : (a + 1) * MSG
            ]
    return out.astype(np.float32)
